# revision 30
# baseline (speedup 1.0000x reference)
import os

import numpy as np
import ml_dtypes

B = 512
H = W = 112
HW = H * W
NCORES = 8
BPC = B // NCORES
P = 128
HALF = HW // 2
QROW = HW // 4
SLOTS = 32
CAP = NCORES * SLOTS

CHUNKS_MASK = [392, 392, 560, 560, 616, 616]
assert sum(CHUNKS_MASK) == QROW
GROUPS_MASK = [3, 3]
CHUNKS_FULL = [784, 784, 1120, 1120, 1232, 1232]
assert sum(CHUNKS_FULL) == HALF
GROUPS_FULL = [3, 3]
ACT_FRAC = 0.72

AB_CHUNKS = [560, 784, 896, 896]
assert sum(AB_CHUNKS) == QROW
AB_GROUPS = [2, 2]
ESLOTS = 16
EREPL = 8
ECOLS = HW // EREPL
EC_CHUNKS = [672, 896]
assert sum(EC_CHUNKS) == ECOLS
ECAP = NCORES * ESLOTS
ACT_FRAC5 = 0.71

NBLK = 12
MODE = os.environ.get("KERNEL_MODE", "v5")

_NC_CACHE = {}


def _emit_chain(nc, mybir, sm, smt):
    f32 = mybir.dt.float32
    AF = mybir.ActivationFunctionType

    r = smt[:, 0:2 * NBLK].rearrange("p (b c) -> p b c", b=2)
    X0 = r[:, :, 0:6:2]
    X1 = r[:, :, 1:6:2]
    YC = r[:, :, 6:9]
    COND = r[:, :, 9:10]

    Dt = sm.tile([P, 6], f32)
    Dr = Dt[:, 0:6].rearrange("p (b c) -> p b c", b=2)
    nc.gpsimd.tensor_sub(Dr, X1, X0)
    SDt = sm.tile([P, 6], f32)
    SDr = SDt[:, 0:6].rearrange("p (b c) -> p b c", b=2)
    nc.gpsimd.tensor_mul(SDr, Dr, YC)
    Et = sm.tile([P, 6], f32)
    Er = Et[:, 0:6].rearrange("p (b c) -> p b c", b=2)
    nc.scalar.activation(out=Er, in_=SDr, func=AF.Exp)
    CE3 = sm.tile([P, 6], f32)
    CE3r = CE3[:, 0:6].rearrange("p (b c) -> p b c", b=2)
    nc.scalar.activation(out=CE3r, in_=Er, func=AF.Ln, bias=1.0)

    tv = sm.tile([P, 2], f32)
    tvr = tv[:, 0:2].rearrange("p (b c) -> p b c", b=2)
    nc.gpsimd.tensor_mul(tvr, COND, Er[:, :, 0:1])
    nc.gpsimd.tensor_scalar_add(tv, tv, 1.0)
    q = sm.tile([P, 1], f32)
    nc.gpsimd.tensor_add(q, CE3[:, 0:1], CE3[:, 1:2])
    nc.gpsimd.tensor_add(q, q, CE3[:, 2:3])
    return tv, q


def _emit_chain_tail(nc, mybir, sm, smt, tv, q):
    f32 = mybir.dt.float32
    OP = mybir.AluOpType
    wv = sm.tile([P, 2], f32)
    nc.vector.reciprocal(wv, tv)
    cepart = sm.tile([P, 1], f32)
    nc.vector.scalar_tensor_tensor(out=cepart, in0=q, scalar=1.0 / (4 * B),
                                   in1=wv[:, 0:1], op0=OP.mult, op1=OP.mult)
    coef_er = sm.tile([P, 1], f32)
    nc.vector.scalar_tensor_tensor(out=coef_er, in0=wv[:, 1:2],
                                   scalar=1.0 / (B * HW),
                                   in1=smt[:, NBLK + 10:NBLK + 11],
                                   op0=OP.mult, op1=OP.mult)
    coef_sp = smt[:, NBLK + 11:NBLK + 12]
    return cepart, coef_er, coef_sp


def _build_nc(masked):
    import concourse.bacc as bacc
    import concourse.tile as tile
    from concourse import mybir

    import bass_rust
    from concourse.hw_specs import get_activation_tables

    f32 = mybir.dt.float32
    bf16 = mybir.dt.bfloat16
    AF = mybir.ActivationFunctionType
    AX = mybir.AxisListType

    chunks = CHUNKS_MASK if masked else CHUNKS_FULL
    groups = GROUPS_MASK if masked else GROUPS_FULL
    row = QROW if masked else HALF
    nchunk = len(chunks)

    nc = bacc.Bacc("TRN2", target_bir_lowering=False, debug=False,
                   num_devices=NCORES)
    act_set_id = list(get_activation_tables("gen3").keys()).index(
        "natural_log_exp_and_others")

    abc = nc.dram_tensor("abc", [P, 3 * row], bf16, kind="ExternalInput").ap()
    small = nc.dram_tensor("small", [P, 2 * NBLK], f32,
                           kind="ExternalInput").ap()
    outp = nc.dram_tensor("out", [1, 1], f32, kind="ExternalOutput").ap()

    gcols = []
    chunk_group = []
    ci = 0
    for g, ng in enumerate(groups):
        off = 0
        for _ in range(ng):
            chunk_group.append((g, off))
            off += chunks[ci]
            ci += 1
        gcols.append(off)
    ngrp = len(groups)

    with tile.TileContext(nc) as tc:
        with (
            tc.tile_pool(name="big", bufs=nchunk) as big,
            tc.tile_pool(name="grp", bufs=ngrp) as grp,
            tc.tile_pool(name="jk", bufs=2) as jk,
            tc.tile_pool(name="sm", bufs=1) as sm,
            tc.tile_pool(name="ps", bufs=1, space="PSUM") as ps,
        ):
            smt = sm.tile([P, 2 * NBLK], f32)
            nc.scalar.dma_start(out=smt, in_=small)
            nc.scalar.add_instruction(bass_rust.InstLoadActFuncSet(
                name=nc.get_next_instruction_name(),
                engine=mybir.EngineType.Activation,
                act_func_set_id=act_set_id,
            ))
            ones = sm.tile([P, 1], f32)
            nc.vector.memset(ones, 1.0)

            abct = []
            off = 0
            for ci, cf in enumerate(chunks):
                t = big.tile([P, 3 * cf], bf16, tag="abct", name=f"abct{ci}")
                nc.sync.dma_start(out=t, in_=abc[:, 3 * off:3 * (off + cf)])
                abct.append(t)
                off += cf

            dg = [grp.tile([P, gcols[g]], bf16, tag="dg", name=f"dg{g}")
                  for g in range(ngrp)]
            eg = [grp.tile([P, gcols[g]], bf16, tag="eg", name=f"eg{g}")
                  for g in range(ngrp)]

            tv, q = _emit_chain(nc, mybir, sm, smt)
            cepart, coef_er, coef_sp = _emit_chain_tail(nc, mybir, sm, smt,
                                                        tv, q)

            for ci, cf in enumerate(chunks):
                t = abct[ci]
                g, og = chunk_group[ci]
                at = t[:, 0:cf]
                nc.vector.tensor_sub(dg[g][:, og:og + cf], at,
                                     t[:, cf:2 * cf])
                nc.vector.tensor_sub(eg[g][:, og:og + cf], at,
                                     t[:, 2 * cf:3 * cf])

            acc = sm.tile([P, 4 * ngrp], f32)
            so = 2 * ngrp
            for g in range(ngrp):
                gc = gcols[g]
                xg = int(gc * ACT_FRAC)
                jd = jk.tile([P, gc], bf16, tag="jact", name=f"jd{g}")
                nc.scalar.activation(out=jd[:, 0:xg], in_=dg[g][:, 0:xg],
                                     func=AF.Square,
                                     accum_out=acc[:, 2 * g:2 * g + 1])
                ja = jk.tile([P, gc], bf16, tag="jamr", name=f"ja{g}")
                nc.vector.affine_mul_reduce(
                    out=ja[:, xg:gc], accum_out=acc[:, 2 * g + 1:2 * g + 2],
                    in0=dg[g][:, xg:gc], in1=dg[g][:, xg:gc],
                    scale=1.0, bias=0.0)
                jd2 = jk.tile([P, gc], bf16, tag="jact", name=f"jd2{g}")
                nc.scalar.activation(out=jd2[:, 0:xg], in_=eg[g][:, 0:xg],
                                     func=AF.Square,
                                     accum_out=acc[:, so + 2 * g:so + 2 * g + 1])
                ja2 = jk.tile([P, gc], bf16, tag="jamr", name=f"ja2{g}")
                nc.vector.affine_mul_reduce(
                    out=ja2[:, xg:gc],
                    accum_out=acc[:, so + 2 * g + 1:so + 2 * g + 2],
                    in0=eg[g][:, xg:gc], in1=eg[g][:, xg:gc],
                    scale=1.0, bias=0.0)

            pt = ps.tile([1, 4 * ngrp + 1], f32)
            nc.tensor.matmul(out=pt[:, 0:so], lhsT=coef_er, rhs=acc[:, 0:so],
                             start=True, stop=True)
            nc.tensor.matmul(out=pt[:, so:2 * so], lhsT=coef_sp,
                             rhs=acc[:, so:2 * so], start=True, stop=True)
            nc.tensor.matmul(out=pt[:, 2 * so:2 * so + 1], lhsT=cepart,
                             rhs=ones, start=True, stop=True)

            res_sb = sm.tile([1, 1], f32)
            nc.vector.reduce_sum(res_sb, pt[:, 0:2 * so + 1], axis=AX.X)
            nc.sync.dma_start(out=outp, in_=res_sb)

    nc.compile()
    return nc


def _build_nc_v5(masked):
    import concourse.bacc as bacc
    import concourse.tile as tile
    from concourse import mybir

    import bass_rust
    from concourse.hw_specs import get_activation_tables

    f32 = mybir.dt.float32
    bf16 = mybir.dt.bfloat16
    AF = mybir.ActivationFunctionType
    AX = mybir.AxisListType

    assert masked
    nc = bacc.Bacc("TRN2", target_bir_lowering=False, debug=False,
                   num_devices=NCORES)
    act_set_id = list(get_activation_tables("gen3").keys()).index(
        "natural_log_exp_and_others")

    ab = nc.dram_tensor("ab", [P, 2 * QROW], bf16, kind="ExternalInput").ap()
    ec = nc.dram_tensor("ec", [P, 2 * ECOLS], bf16, kind="ExternalInput").ap()
    small = nc.dram_tensor("small", [P, 2 * NBLK], f32,
                           kind="ExternalInput").ap()
    outp = nc.dram_tensor("out", [1, 1], f32, kind="ExternalOutput").ap()

    nab = len(AB_CHUNKS)
    nec = len(EC_CHUNKS)
    ngrp = len(AB_GROUPS)
    ab_group = []
    gcols = []
    ci = 0
    for g, ng in enumerate(AB_GROUPS):
        off = 0
        for _ in range(ng):
            ab_group.append((g, off))
            off += AB_CHUNKS[ci]
            ci += 1
        gcols.append(off)

    with tile.TileContext(nc) as tc:
        with (
            tc.tile_pool(name="big", bufs=nab) as big,
            tc.tile_pool(name="bec", bufs=nec) as bec,
            tc.tile_pool(name="grp", bufs=ngrp) as grp,
            tc.tile_pool(name="egr", bufs=nec) as egr,
            tc.tile_pool(name="jk", bufs=2) as jk,
            tc.tile_pool(name="sm", bufs=1) as sm,
            tc.tile_pool(name="ps", bufs=1, space="PSUM") as ps,
        ):
            smt = sm.tile([P, 2 * NBLK], f32)
            nc.scalar.dma_start(out=smt, in_=small)
            ect = []
            off = 0
            for ci, cf in enumerate(EC_CHUNKS):
                t = bec.tile([P, 2 * cf], bf16, tag="ect", name=f"ect{ci}")
                nc.scalar.dma_start(out=t, in_=ec[:, 2 * off:2 * (off + cf)])
                ect.append(t)
                off += cf
            nc.scalar.add_instruction(bass_rust.InstLoadActFuncSet(
                name=nc.get_next_instruction_name(),
                engine=mybir.EngineType.Activation,
                act_func_set_id=act_set_id,
            ))
            ones = sm.tile([P, 1], f32)
            nc.vector.memset(ones, 1.0)

            abt = []
            off = 0
            for ci, cf in enumerate(AB_CHUNKS):
                t = big.tile([P, 2 * cf], bf16, tag="abt", name=f"abt{ci}")
                nc.sync.dma_start(out=t, in_=ab[:, 2 * off:2 * (off + cf)])
                abt.append(t)
                off += cf

            dg = [grp.tile([P, gcols[g]], bf16, tag="dg", name=f"dg{g}")
                  for g in range(ngrp)]
            et = [egr.tile([P, EC_CHUNKS[i]], bf16, tag="et", name=f"et{i}")
                  for i in range(nec)]

            tv, q = _emit_chain(nc, mybir, sm, smt)

            def sub_d(ci):
                cf = AB_CHUNKS[ci]
                g, og = ab_group[ci]
                t = abt[ci]
                nc.vector.tensor_sub(dg[g][:, og:og + cf], t[:, 0:cf],
                                     t[:, cf:2 * cf])

            def sub_e(ci):
                cf = EC_CHUNKS[ci]
                t = ect[ci]
                nc.vector.tensor_sub(et[ci], t[:, 0:cf], t[:, cf:2 * cf])

            acc = sm.tile([P, 2 * ngrp + 2 * nec], f32)
            so = 2 * ngrp

            def sq_d(g):
                gc = gcols[g]
                xg = int(gc * ACT_FRAC5)
                jd = jk.tile([P, gc], bf16, tag="jact", name=f"jkd{g}")
                nc.scalar.activation(out=jd[:, 0:xg], in_=dg[g][:, 0:xg],
                                     func=AF.Square,
                                     accum_out=acc[:, 2 * g:2 * g + 1])

            def amr_d(g):
                gc = gcols[g]
                xg = int(gc * ACT_FRAC5)
                ja = jk.tile([P, gc], bf16, tag="jamr", name=f"jad{g}")
                nc.vector.affine_mul_reduce(
                    out=ja[:, xg:gc], accum_out=acc[:, 2 * g + 1:2 * g + 2],
                    in0=dg[g][:, xg:gc], in1=dg[g][:, xg:gc],
                    scale=1.0, bias=0.0)

            def sq_e(ci):
                cf = EC_CHUNKS[ci]
                xe = int(cf * ACT_FRAC5)
                je = jk.tile([P, cf], bf16, tag="jact", name=f"jke{ci}")
                nc.scalar.activation(
                    out=je[:, 0:xe], in_=et[ci][:, 0:xe], func=AF.Square,
                    accum_out=acc[:, so + 2 * ci:so + 2 * ci + 1])

            def amr_e(ci):
                cf = EC_CHUNKS[ci]
                xe = int(cf * ACT_FRAC5)
                ja = jk.tile([P, cf], bf16, tag="jamr", name=f"jae{ci}")
                nc.vector.affine_mul_reduce(
                    out=ja[:, xe:cf],
                    accum_out=acc[:, so + 2 * ci + 1:so + 2 * ci + 2],
                    in0=et[ci][:, xe:cf], in1=et[ci][:, xe:cf],
                    scale=1.0, bias=0.0)

            sub_d(0)
            sub_d(1)
            sub_d(2)
            sub_e(0)
            sub_d(3)
            sub_e(1)
            sq_d(0)
            amr_d(0)
            sq_e(0)
            amr_e(0)
            sq_d(1)
            amr_d(1)
            sq_e(1)
            amr_e(1)

            cepart, coef_er, coef_sp = _emit_chain_tail(nc, mybir, sm, smt,
                                                        tv, q)

            ner = 2 * ngrp
            nsp = 2 * nec
            pt = ps.tile([1, ner + nsp + 1], f32)
            nc.tensor.matmul(out=pt[:, 0:ner], lhsT=coef_er,
                             rhs=acc[:, 0:ner], start=True, stop=True)
            nc.tensor.matmul(out=pt[:, ner:ner + nsp], lhsT=coef_sp,
                             rhs=acc[:, ner:ner + nsp], start=True, stop=True)
            nc.tensor.matmul(out=pt[:, ner + nsp:ner + nsp + 1],
                             lhsT=cepart, rhs=ones, start=True, stop=True)

            res_sb = sm.tile([1, 1], f32)
            nc.vector.reduce_sum(res_sb, pt[:, 0:ner + nsp + 1], axis=AX.X)
            nc.sync.dma_start(out=outp, in_=res_sb)

    nc.compile()
    return nc


def _get_nc_mode(masked, mode):
    key = ("mask" if masked else "full") + mode
    if key not in _NC_CACHE:
        builder = _build_nc_v5 if (mode == "v5" and masked) else _build_nc
        _NC_CACHE[key] = builder(masked)
    return _NC_CACHE[key]


def _interleave2(x, y, chunks):
    row = x.shape[1]
    out = np.empty((P, 2 * row), dtype=np.float32)
    off = 0
    for cf in chunks:
        sl = slice(off, off + cf)
        out[:, 2 * off:2 * off + cf] = x[:, sl]
        out[:, 2 * off + cf:2 * off + 2 * cf] = y[:, sl]
        off += cf
    return out.astype(ml_dtypes.bfloat16)


def _interleave3(a, b, c, chunks):
    row = a.shape[1]
    abc = np.empty((P, 3 * row), dtype=np.float32)
    off = 0
    for cf in chunks:
        sl = slice(off, off + cf)
        abc[:, 3 * off:3 * off + cf] = a[:, sl]
        abc[:, 3 * off + cf:3 * off + 2 * cf] = b[:, sl]
        abc[:, 3 * off + 2 * cf:3 * off + 3 * cf] = c[:, sl]
        off += cf
    return abc.astype(ml_dtypes.bfloat16)


def _small_block(p1i, p1o, p2i, pbi, y):
    n = len(y)
    blk = np.zeros((n, NBLK), dtype=np.float32)
    yf = y.astype(np.float32)
    blk[:, 0:2] = p1i
    blk[:, 2:4] = p2i
    pb = pbi.copy()
    pb[y == 0] = np.array([0.0, -100.0], dtype=np.float32)
    blk[:, 4:6] = pb
    blk[:, 6] = 1.0 - 2.0 * yf
    blk[:, 7] = 1.0 - 2.0 * yf
    blk[:, 8] = 1.0
    cur = (p1i[:, 1] > p1i[:, 0])
    flag = (p1o[:, 1] > p1o[:, 0])
    cond = (cur != flag) & (~cur) & (y == 1)
    blk[:, 9] = cond.astype(np.float32)
    same = (cur == flag).astype(np.float32)
    blk[:, 10] = yf
    blk[:, 11] = yf * same / (B * HW)
    return blk


def _small_cam_block(p1i, p1o, y, repl):
    n = len(y)
    blk = np.zeros((n, NBLK), dtype=np.float32)
    yf = y.astype(np.float32)
    blk[:, 0:2] = p1i
    blk[:, 6] = -1.0
    cur = (p1i[:, 1] > p1i[:, 0])
    flag = (p1o[:, 1] > p1o[:, 0])
    cond = (cur != flag) & (~cur) & (y == 1)
    blk[:, 9] = cond.astype(np.float32)
    same = (cur == flag).astype(np.float32)
    blk[:, 10] = yf
    blk[:, 11] = yf * same / (B * HW)
    return np.repeat(blk, repl, axis=0)


def kernel(preds1, cams1, preds1_back, preds2, cams2, y, index):
    from concourse.bass_utils import run_bass_kernel_spmd

    idx = int(np.asarray(index))
    preds1 = np.asarray(preds1, dtype=np.float32)
    preds1_back = np.asarray(preds1_back, dtype=np.float32)
    preds2 = np.asarray(preds2, dtype=np.float32)
    cams1 = np.asarray(cams1, dtype=np.float32)
    cams2 = np.asarray(cams2, dtype=np.float32)
    yi = np.asarray(y).astype(np.int64).reshape(B)

    sel = np.flatnonzero(yi == 1)
    masked = len(sel) <= CAP
    cur = preds1[idx, :, 1] > preds1[idx, :, 0]
    flag = preds1[1 - idx, :, 1] > preds1[1 - idx, :, 0]
    sel_e = np.flatnonzero((yi == 1) & (cur == flag))
    use_v5 = masked and MODE == "v5" and len(sel_e) <= ECAP
    nc = _get_nc_mode(masked, "v5" if use_v5 else "v3")

    chunks = CHUNKS_MASK if masked else CHUNKS_FULL
    row = QROW if masked else HALF
    slots = SLOTS if masked else BPC
    repl = 4 if masked else 2

    in_maps = []
    for k in range(NCORES):
        s = slice(k * BPC, (k + 1) * BPC)
        ce_blk = _small_block(preds1[idx, s], preds1[1 - idx, s],
                              preds2[idx, s], preds1_back[idx, s], yi[s])
        ce_blk = np.repeat(ce_blk, 2, axis=0)

        if masked:
            sel_k = sel[k * SLOTS:(k + 1) * SLOTS]
            nk = len(sel_k)
            a = np.zeros((slots, HW), dtype=np.float32)
            b = np.zeros((slots, HW), dtype=np.float32)
            a[:nk] = cams1[idx, sel_k, 1].reshape(nk, HW)
            b[:nk] = cams2[idx, sel_k, 1].reshape(nk, HW)
            if not use_v5:
                c = np.zeros((slots, HW), dtype=np.float32)
                c[:nk] = cams1[1 - idx, sel_k, 1].reshape(nk, HW)
            p1i = np.zeros((slots, 2), dtype=np.float32)
            p1o = np.zeros((slots, 2), dtype=np.float32)
            ys = np.zeros(slots, dtype=np.int64)
            p1i[:nk] = preds1[idx, sel_k]
            p1o[:nk] = preds1[1 - idx, sel_k]
            ys[:nk] = yi[sel_k]
            cam_blk = _small_cam_block(p1i, p1o, ys, repl)
        else:
            a = cams1[idx, s, 1].reshape(BPC, HW)
            b = cams2[idx, s, 1].reshape(BPC, HW)
            c = cams1[1 - idx, s, 1].reshape(BPC, HW)
            cam_blk = _small_cam_block(preds1[idx, s], preds1[1 - idx, s],
                                       yi[s], repl)

        if use_v5:
            sel_ek = sel_e[k * ESLOTS:(k + 1) * ESLOTS]
            nke = len(sel_ek)
            ae = np.zeros((ESLOTS, HW), dtype=np.float32)
            ce_ = np.zeros((ESLOTS, HW), dtype=np.float32)
            ae[:nke] = cams1[idx, sel_ek, 1].reshape(nke, HW)
            ce_[:nke] = cams1[1 - idx, sel_ek, 1].reshape(nke, HW)
            spcol = np.zeros(ESLOTS, dtype=np.float32)
            spcol[:nke] = 1.0 / (B * HW)
            cam_blk = cam_blk.copy()
            cam_blk[:, 11] = np.repeat(spcol, EREPL)
            im = {
                "ab": _interleave2(a.reshape(P, QROW), b.reshape(P, QROW),
                                   AB_CHUNKS),
                "ec": _interleave2(ae.reshape(P, ECOLS),
                                   ce_.reshape(P, ECOLS), EC_CHUNKS),
                "small": np.ascontiguousarray(
                    np.concatenate([ce_blk, cam_blk], axis=1)),
            }
        else:
            im = {
                "abc": _interleave3(a.reshape(P, row), b.reshape(P, row),
                                    c.reshape(P, row), chunks),
                "small": np.ascontiguousarray(
                    np.concatenate([ce_blk, cam_blk], axis=1)),
            }
        in_maps.append(im)

    trace = bool(int(os.environ.get("KERNEL_TRACE", "0")))
    res = run_bass_kernel_spmd(nc, in_maps, core_ids=list(range(NCORES)),
                               trace=trace)
    kernel.last_exec_time_ns = res.exec_time_ns
    kernel.last_result = res
    total = sum(float(res.results[k]["out"][0, 0]) for k in range(NCORES))
    return np.array(total, dtype=np.float32)


kernel.last_exec_time_ns = None
kernel.last_result = None


# revision 33
# speedup vs baseline: 1.1327x; 1.1327x over previous
import os

import numpy as np
import ml_dtypes

B = 512
H = W = 112
HW = H * W
NCORES = 8
BPC = B // NCORES
P = 128
HALF = HW // 2
QROW = HW // 4
SLOTS = 32
CAP = NCORES * SLOTS

CHUNKS_MASK = [392, 392, 560, 560, 616, 616]
assert sum(CHUNKS_MASK) == QROW
GROUPS_MASK = [3, 3]
CHUNKS_FULL = [784, 784, 1120, 1120, 1232, 1232]
assert sum(CHUNKS_FULL) == HALF
GROUPS_FULL = [3, 3]
ACT_FRAC = 0.72

AB_CHUNKS = [560, 784, 896, 896]
assert sum(AB_CHUNKS) == QROW
AB_GROUPS = [2, 2]
ESLOTS = 16
EREPL = 8
ECOLS = HW // EREPL
EC_CHUNKS = [672, 896]
assert sum(EC_CHUNKS) == ECOLS
ECAP = NCORES * ESLOTS
ACT_FRAC5 = 0.71

NBLK = 12
MODE = os.environ.get("KERNEL_MODE", "v5")

_NC_CACHE = {}


def _emit_chain(nc, mybir, sm, smt):
    f32 = mybir.dt.float32
    AF = mybir.ActivationFunctionType

    r = smt[:, 0:2 * NBLK].rearrange("p (b c) -> p b c", b=2)
    X0 = r[:, :, 0:6:2]
    X1 = r[:, :, 1:6:2]
    YC = r[:, :, 6:9]
    COND = r[:, :, 9:10]

    Dt = sm.tile([P, 6], f32)
    Dr = Dt[:, 0:6].rearrange("p (b c) -> p b c", b=2)
    nc.gpsimd.tensor_sub(Dr, X1, X0)
    SDt = sm.tile([P, 6], f32)
    SDr = SDt[:, 0:6].rearrange("p (b c) -> p b c", b=2)
    nc.gpsimd.tensor_mul(SDr, Dr, YC)
    Et = sm.tile([P, 6], f32)
    Er = Et[:, 0:6].rearrange("p (b c) -> p b c", b=2)
    nc.scalar.activation(out=Er, in_=SDr, func=AF.Exp)
    CE3 = sm.tile([P, 6], f32)
    CE3r = CE3[:, 0:6].rearrange("p (b c) -> p b c", b=2)
    nc.scalar.activation(out=CE3r, in_=Er, func=AF.Ln, bias=1.0)

    tv = sm.tile([P, 2], f32)
    tvr = tv[:, 0:2].rearrange("p (b c) -> p b c", b=2)
    nc.gpsimd.tensor_mul(tvr, COND, Er[:, :, 0:1])
    nc.gpsimd.tensor_scalar_add(tv, tv, 1.0)
    q = sm.tile([P, 1], f32)
    nc.gpsimd.tensor_add(q, CE3[:, 0:1], CE3[:, 1:2])
    nc.gpsimd.tensor_add(q, q, CE3[:, 2:3])
    return tv, q


def _emit_chain_tail(nc, mybir, sm, smt, tv, q):
    f32 = mybir.dt.float32
    OP = mybir.AluOpType
    wv = sm.tile([P, 2], f32)
    nc.vector.reciprocal(wv, tv)
    cepart = sm.tile([P, 1], f32)
    nc.vector.scalar_tensor_tensor(out=cepart, in0=q, scalar=1.0 / (4 * B),
                                   in1=wv[:, 0:1], op0=OP.mult, op1=OP.mult)
    coef_er = sm.tile([P, 1], f32)
    nc.vector.scalar_tensor_tensor(out=coef_er, in0=wv[:, 1:2],
                                   scalar=1.0 / (B * HW),
                                   in1=smt[:, NBLK + 10:NBLK + 11],
                                   op0=OP.mult, op1=OP.mult)
    coef_sp = smt[:, NBLK + 11:NBLK + 12]
    return cepart, coef_er, coef_sp


def _build_nc(masked):
    import concourse.bacc as bacc
    import concourse.tile as tile
    from concourse import mybir

    import bass_rust
    from concourse.hw_specs import get_activation_tables

    f32 = mybir.dt.float32
    bf16 = mybir.dt.bfloat16
    AF = mybir.ActivationFunctionType
    AX = mybir.AxisListType

    chunks = CHUNKS_MASK if masked else CHUNKS_FULL
    groups = GROUPS_MASK if masked else GROUPS_FULL
    row = QROW if masked else HALF
    nchunk = len(chunks)

    nc = bacc.Bacc("TRN2", target_bir_lowering=False, debug=False,
                   num_devices=NCORES)
    act_set_id = list(get_activation_tables("gen3").keys()).index(
        "natural_log_exp_and_others")

    abc = nc.dram_tensor("abc", [P, 3 * row], bf16, kind="ExternalInput").ap()
    small = nc.dram_tensor("small", [P, 2 * NBLK], f32,
                           kind="ExternalInput").ap()
    outp = nc.dram_tensor("out", [1, 1], f32, kind="ExternalOutput").ap()

    gcols = []
    chunk_group = []
    ci = 0
    for g, ng in enumerate(groups):
        off = 0
        for _ in range(ng):
            chunk_group.append((g, off))
            off += chunks[ci]
            ci += 1
        gcols.append(off)
    ngrp = len(groups)

    with tile.TileContext(nc) as tc:
        with (
            tc.tile_pool(name="big", bufs=nchunk) as big,
            tc.tile_pool(name="grp", bufs=ngrp) as grp,
            tc.tile_pool(name="jk", bufs=2) as jk,
            tc.tile_pool(name="sm", bufs=1) as sm,
            tc.tile_pool(name="ps", bufs=1, space="PSUM") as ps,
        ):
            smt = sm.tile([P, 2 * NBLK], f32)
            nc.scalar.dma_start(out=smt, in_=small)
            nc.scalar.add_instruction(bass_rust.InstLoadActFuncSet(
                name=nc.get_next_instruction_name(),
                engine=mybir.EngineType.Activation,
                act_func_set_id=act_set_id,
            ))
            ones = sm.tile([P, 1], f32)
            nc.vector.memset(ones, 1.0)

            abct = []
            off = 0
            for ci, cf in enumerate(chunks):
                t = big.tile([P, 3 * cf], bf16, tag="abct", name=f"abct{ci}")
                nc.sync.dma_start(out=t, in_=abc[:, 3 * off:3 * (off + cf)])
                abct.append(t)
                off += cf

            dg = [grp.tile([P, gcols[g]], bf16, tag="dg", name=f"dg{g}")
                  for g in range(ngrp)]
            eg = [grp.tile([P, gcols[g]], bf16, tag="eg", name=f"eg{g}")
                  for g in range(ngrp)]

            tv, q = _emit_chain(nc, mybir, sm, smt)
            cepart, coef_er, coef_sp = _emit_chain_tail(nc, mybir, sm, smt,
                                                        tv, q)

            for ci, cf in enumerate(chunks):
                t = abct[ci]
                g, og = chunk_group[ci]
                at = t[:, 0:cf]
                nc.vector.tensor_sub(dg[g][:, og:og + cf], at,
                                     t[:, cf:2 * cf])
                nc.vector.tensor_sub(eg[g][:, og:og + cf], at,
                                     t[:, 2 * cf:3 * cf])

            acc = sm.tile([P, 4 * ngrp], f32)
            so = 2 * ngrp
            for g in range(ngrp):
                gc = gcols[g]
                xg = int(gc * ACT_FRAC)
                jd = jk.tile([P, gc], bf16, tag="jact", name=f"jd{g}")
                nc.scalar.activation(out=jd[:, 0:xg], in_=dg[g][:, 0:xg],
                                     func=AF.Square,
                                     accum_out=acc[:, 2 * g:2 * g + 1])
                ja = jk.tile([P, gc], bf16, tag="jamr", name=f"ja{g}")
                nc.vector.affine_mul_reduce(
                    out=ja[:, xg:gc], accum_out=acc[:, 2 * g + 1:2 * g + 2],
                    in0=dg[g][:, xg:gc], in1=dg[g][:, xg:gc],
                    scale=1.0, bias=0.0)
                jd2 = jk.tile([P, gc], bf16, tag="jact", name=f"jd2{g}")
                nc.scalar.activation(out=jd2[:, 0:xg], in_=eg[g][:, 0:xg],
                                     func=AF.Square,
                                     accum_out=acc[:, so + 2 * g:so + 2 * g + 1])
                ja2 = jk.tile([P, gc], bf16, tag="jamr", name=f"ja2{g}")
                nc.vector.affine_mul_reduce(
                    out=ja2[:, xg:gc],
                    accum_out=acc[:, so + 2 * g + 1:so + 2 * g + 2],
                    in0=eg[g][:, xg:gc], in1=eg[g][:, xg:gc],
                    scale=1.0, bias=0.0)

            pt = ps.tile([1, 4 * ngrp + 1], f32)
            nc.tensor.matmul(out=pt[:, 0:so], lhsT=coef_er, rhs=acc[:, 0:so],
                             start=True, stop=True)
            nc.tensor.matmul(out=pt[:, so:2 * so], lhsT=coef_sp,
                             rhs=acc[:, so:2 * so], start=True, stop=True)
            nc.tensor.matmul(out=pt[:, 2 * so:2 * so + 1], lhsT=cepart,
                             rhs=ones, start=True, stop=True)

            res_sb = sm.tile([1, 1], f32)
            nc.vector.reduce_sum(res_sb, pt[:, 0:2 * so + 1], axis=AX.X)
            nc.sync.dma_start(out=outp, in_=res_sb)

    nc.compile()
    return nc


def _build_nc_v5(masked):
    import concourse.bacc as bacc
    import concourse.tile as tile
    from concourse import mybir

    import bass_rust
    from concourse.hw_specs import get_activation_tables

    f32 = mybir.dt.float32
    bf16 = mybir.dt.bfloat16
    AF = mybir.ActivationFunctionType
    AX = mybir.AxisListType

    assert masked
    nc = bacc.Bacc("TRN2", target_bir_lowering=False, debug=False,
                   num_devices=NCORES)
    act_set_id = list(get_activation_tables("gen3").keys()).index(
        "natural_log_exp_and_others")

    ab = nc.dram_tensor("ab", [P, 2 * QROW], bf16, kind="ExternalInput").ap()
    ec = nc.dram_tensor("ec", [P, 2 * ECOLS], bf16, kind="ExternalInput").ap()
    small = nc.dram_tensor("small", [P, 2 * NBLK], f32,
                           kind="ExternalInput").ap()
    outp = nc.dram_tensor("out", [1, 1], f32, kind="ExternalOutput").ap()

    nab = len(AB_CHUNKS)
    nec = len(EC_CHUNKS)
    ngrp = len(AB_GROUPS)
    ab_group = []
    gcols = []
    ci = 0
    for g, ng in enumerate(AB_GROUPS):
        off = 0
        for _ in range(ng):
            ab_group.append((g, off))
            off += AB_CHUNKS[ci]
            ci += 1
        gcols.append(off)

    with tile.TileContext(nc) as tc:
        with (
            tc.tile_pool(name="big", bufs=nab) as big,
            tc.tile_pool(name="bec", bufs=nec) as bec,
            tc.tile_pool(name="grp", bufs=ngrp) as grp,
            tc.tile_pool(name="egr", bufs=nec) as egr,
            tc.tile_pool(name="jk", bufs=2) as jk,
            tc.tile_pool(name="sm", bufs=1) as sm,
            tc.tile_pool(name="ps", bufs=1, space="PSUM") as ps,
        ):
            smt = sm.tile([P, 2 * NBLK], f32)
            nc.scalar.dma_start(out=smt, in_=small)
            nc.scalar.add_instruction(bass_rust.InstLoadActFuncSet(
                name=nc.get_next_instruction_name(),
                engine=mybir.EngineType.Activation,
                act_func_set_id=act_set_id,
            ))
            ones = sm.tile([P, 1], f32)
            nc.vector.memset(ones, 1.0)

            abt = [None] * nab
            ect = [None] * nec
            ab_off = [0] * nab
            off = 0
            for ci, cf in enumerate(AB_CHUNKS):
                ab_off[ci] = off
                off += cf
            ec_off = [0] * nec
            off = 0
            for ci, cf in enumerate(EC_CHUNKS):
                ec_off[ci] = off
                off += cf

            def dma_ab(ci):
                cf = AB_CHUNKS[ci]
                o = ab_off[ci]
                t = big.tile([P, 2 * cf], bf16, tag="abt", name=f"abt{ci}")
                nc.sync.dma_start(out=t, in_=ab[:, 2 * o:2 * (o + cf)])
                abt[ci] = t

            def dma_ec(ci):
                cf = EC_CHUNKS[ci]
                o = ec_off[ci]
                t = bec.tile([P, 2 * cf], bf16, tag="ect", name=f"ect{ci}")
                nc.sync.dma_start(out=t, in_=ec[:, 2 * o:2 * (o + cf)])
                ect[ci] = t

            dma_ab(0)
            dma_ab(1)
            dma_ec(0)
            dma_ab(2)
            dma_ec(1)
            dma_ab(3)

            dg = [grp.tile([P, gcols[g]], bf16, tag="dg", name=f"dg{g}")
                  for g in range(ngrp)]
            et = [egr.tile([P, EC_CHUNKS[i]], bf16, tag="et", name=f"et{i}")
                  for i in range(nec)]

            tv, q = _emit_chain(nc, mybir, sm, smt)

            def sub_d(ci):
                cf = AB_CHUNKS[ci]
                g, og = ab_group[ci]
                t = abt[ci]
                nc.vector.tensor_sub(dg[g][:, og:og + cf], t[:, 0:cf],
                                     t[:, cf:2 * cf])

            def sub_e(ci):
                cf = EC_CHUNKS[ci]
                t = ect[ci]
                nc.vector.tensor_sub(et[ci], t[:, 0:cf], t[:, cf:2 * cf])

            acc = sm.tile([P, 2 * ngrp + 2 * nec], f32)
            so = 2 * ngrp

            def sq_d(g):
                gc = gcols[g]
                xg = int(gc * ACT_FRAC5)
                jd = jk.tile([P, gc], bf16, tag="jact", name=f"jkd{g}")
                nc.scalar.activation(out=jd[:, 0:xg], in_=dg[g][:, 0:xg],
                                     func=AF.Square,
                                     accum_out=acc[:, 2 * g:2 * g + 1])

            def amr_d(g):
                gc = gcols[g]
                xg = int(gc * ACT_FRAC5)
                ja = jk.tile([P, gc], bf16, tag="jamr", name=f"jad{g}")
                nc.vector.affine_mul_reduce(
                    out=ja[:, xg:gc], accum_out=acc[:, 2 * g + 1:2 * g + 2],
                    in0=dg[g][:, xg:gc], in1=dg[g][:, xg:gc],
                    scale=1.0, bias=0.0)

            def sq_e(ci):
                cf = EC_CHUNKS[ci]
                xe = int(cf * ACT_FRAC5)
                je = jk.tile([P, cf], bf16, tag="jact", name=f"jke{ci}")
                nc.scalar.activation(
                    out=je[:, 0:xe], in_=et[ci][:, 0:xe], func=AF.Square,
                    accum_out=acc[:, so + 2 * ci:so + 2 * ci + 1])

            def amr_e(ci):
                cf = EC_CHUNKS[ci]
                xe = int(cf * ACT_FRAC5)
                ja = jk.tile([P, cf], bf16, tag="jamr", name=f"jae{ci}")
                nc.vector.affine_mul_reduce(
                    out=ja[:, xe:cf],
                    accum_out=acc[:, so + 2 * ci + 1:so + 2 * ci + 2],
                    in0=et[ci][:, xe:cf], in1=et[ci][:, xe:cf],
                    scale=1.0, bias=0.0)

            sub_d(0)
            sub_d(1)
            sub_e(0)
            sub_d(2)
            sub_e(1)
            sub_d(3)
            sq_d(0)
            amr_d(0)
            sq_e(0)
            amr_e(0)
            sq_e(1)
            amr_e(1)
            sq_d(1)
            amr_d(1)

            cepart, coef_er, coef_sp = _emit_chain_tail(nc, mybir, sm, smt,
                                                        tv, q)

            ner = 2 * ngrp
            nsp = 2 * nec
            pt = ps.tile([1, ner + nsp + 1], f32)
            nc.tensor.matmul(out=pt[:, 0:ner], lhsT=coef_er,
                             rhs=acc[:, 0:ner], start=True, stop=True)
            nc.tensor.matmul(out=pt[:, ner:ner + nsp], lhsT=coef_sp,
                             rhs=acc[:, ner:ner + nsp], start=True, stop=True)
            nc.tensor.matmul(out=pt[:, ner + nsp:ner + nsp + 1],
                             lhsT=cepart, rhs=ones, start=True, stop=True)

            res_sb = sm.tile([1, 1], f32)
            nc.vector.reduce_sum(res_sb, pt[:, 0:ner + nsp + 1], axis=AX.X)
            nc.sync.dma_start(out=outp, in_=res_sb)

    nc.compile()
    return nc


def _get_nc_mode(masked, mode):
    key = ("mask" if masked else "full") + mode
    if key not in _NC_CACHE:
        builder = _build_nc_v5 if (mode == "v5" and masked) else _build_nc
        _NC_CACHE[key] = builder(masked)
    return _NC_CACHE[key]


def _interleave2(x, y, chunks):
    row = x.shape[1]
    out = np.empty((P, 2 * row), dtype=np.float32)
    off = 0
    for cf in chunks:
        sl = slice(off, off + cf)
        out[:, 2 * off:2 * off + cf] = x[:, sl]
        out[:, 2 * off + cf:2 * off + 2 * cf] = y[:, sl]
        off += cf
    return out.astype(ml_dtypes.bfloat16)


def _interleave3(a, b, c, chunks):
    row = a.shape[1]
    abc = np.empty((P, 3 * row), dtype=np.float32)
    off = 0
    for cf in chunks:
        sl = slice(off, off + cf)
        abc[:, 3 * off:3 * off + cf] = a[:, sl]
        abc[:, 3 * off + cf:3 * off + 2 * cf] = b[:, sl]
        abc[:, 3 * off + 2 * cf:3 * off + 3 * cf] = c[:, sl]
        off += cf
    return abc.astype(ml_dtypes.bfloat16)


def _small_block(p1i, p1o, p2i, pbi, y):
    n = len(y)
    blk = np.zeros((n, NBLK), dtype=np.float32)
    yf = y.astype(np.float32)
    blk[:, 0:2] = p1i
    blk[:, 2:4] = p2i
    pb = pbi.copy()
    pb[y == 0] = np.array([0.0, -100.0], dtype=np.float32)
    blk[:, 4:6] = pb
    blk[:, 6] = 1.0 - 2.0 * yf
    blk[:, 7] = 1.0 - 2.0 * yf
    blk[:, 8] = 1.0
    cur = (p1i[:, 1] > p1i[:, 0])
    flag = (p1o[:, 1] > p1o[:, 0])
    cond = (cur != flag) & (~cur) & (y == 1)
    blk[:, 9] = cond.astype(np.float32)
    same = (cur == flag).astype(np.float32)
    blk[:, 10] = yf
    blk[:, 11] = yf * same / (B * HW)
    return blk


def _small_cam_block(p1i, p1o, y, repl):
    n = len(y)
    blk = np.zeros((n, NBLK), dtype=np.float32)
    yf = y.astype(np.float32)
    blk[:, 0:2] = p1i
    blk[:, 6] = -1.0
    cur = (p1i[:, 1] > p1i[:, 0])
    flag = (p1o[:, 1] > p1o[:, 0])
    cond = (cur != flag) & (~cur) & (y == 1)
    blk[:, 9] = cond.astype(np.float32)
    same = (cur == flag).astype(np.float32)
    blk[:, 10] = yf
    blk[:, 11] = yf * same / (B * HW)
    return np.repeat(blk, repl, axis=0)


def kernel(preds1, cams1, preds1_back, preds2, cams2, y, index):
    from concourse.bass_utils import run_bass_kernel_spmd

    idx = int(np.asarray(index))
    preds1 = np.asarray(preds1, dtype=np.float32)
    preds1_back = np.asarray(preds1_back, dtype=np.float32)
    preds2 = np.asarray(preds2, dtype=np.float32)
    cams1 = np.asarray(cams1, dtype=np.float32)
    cams2 = np.asarray(cams2, dtype=np.float32)
    yi = np.asarray(y).astype(np.int64).reshape(B)

    sel = np.flatnonzero(yi == 1)
    masked = len(sel) <= CAP
    cur = preds1[idx, :, 1] > preds1[idx, :, 0]
    flag = preds1[1 - idx, :, 1] > preds1[1 - idx, :, 0]
    sel_e = np.flatnonzero((yi == 1) & (cur == flag))
    use_v5 = masked and MODE == "v5" and len(sel_e) <= ECAP
    nc = _get_nc_mode(masked, "v5" if use_v5 else "v3")

    chunks = CHUNKS_MASK if masked else CHUNKS_FULL
    row = QROW if masked else HALF
    slots = SLOTS if masked else BPC
    repl = 4 if masked else 2

    in_maps = []
    for k in range(NCORES):
        s = slice(k * BPC, (k + 1) * BPC)
        ce_blk = _small_block(preds1[idx, s], preds1[1 - idx, s],
                              preds2[idx, s], preds1_back[idx, s], yi[s])
        ce_blk = np.repeat(ce_blk, 2, axis=0)

        if masked:
            sel_k = sel[k * SLOTS:(k + 1) * SLOTS]
            nk = len(sel_k)
            a = np.zeros((slots, HW), dtype=np.float32)
            b = np.zeros((slots, HW), dtype=np.float32)
            a[:nk] = cams1[idx, sel_k, 1].reshape(nk, HW)
            b[:nk] = cams2[idx, sel_k, 1].reshape(nk, HW)
            if not use_v5:
                c = np.zeros((slots, HW), dtype=np.float32)
                c[:nk] = cams1[1 - idx, sel_k, 1].reshape(nk, HW)
            p1i = np.zeros((slots, 2), dtype=np.float32)
            p1o = np.zeros((slots, 2), dtype=np.float32)
            ys = np.zeros(slots, dtype=np.int64)
            p1i[:nk] = preds1[idx, sel_k]
            p1o[:nk] = preds1[1 - idx, sel_k]
            ys[:nk] = yi[sel_k]
            cam_blk = _small_cam_block(p1i, p1o, ys, repl)
        else:
            a = cams1[idx, s, 1].reshape(BPC, HW)
            b = cams2[idx, s, 1].reshape(BPC, HW)
            c = cams1[1 - idx, s, 1].reshape(BPC, HW)
            cam_blk = _small_cam_block(preds1[idx, s], preds1[1 - idx, s],
                                       yi[s], repl)

        if use_v5:
            sel_ek = sel_e[k * ESLOTS:(k + 1) * ESLOTS]
            nke = len(sel_ek)
            ae = np.zeros((ESLOTS, HW), dtype=np.float32)
            ce_ = np.zeros((ESLOTS, HW), dtype=np.float32)
            ae[:nke] = cams1[idx, sel_ek, 1].reshape(nke, HW)
            ce_[:nke] = cams1[1 - idx, sel_ek, 1].reshape(nke, HW)
            spcol = np.zeros(ESLOTS, dtype=np.float32)
            spcol[:nke] = 1.0 / (B * HW)
            cam_blk = cam_blk.copy()
            cam_blk[:, 11] = np.repeat(spcol, EREPL)
            im = {
                "ab": _interleave2(a.reshape(P, QROW), b.reshape(P, QROW),
                                   AB_CHUNKS),
                "ec": _interleave2(ae.reshape(P, ECOLS),
                                   ce_.reshape(P, ECOLS), EC_CHUNKS),
                "small": np.ascontiguousarray(
                    np.concatenate([ce_blk, cam_blk], axis=1)),
            }
        else:
            im = {
                "abc": _interleave3(a.reshape(P, row), b.reshape(P, row),
                                    c.reshape(P, row), chunks),
                "small": np.ascontiguousarray(
                    np.concatenate([ce_blk, cam_blk], axis=1)),
            }
        in_maps.append(im)

    trace = bool(int(os.environ.get("KERNEL_TRACE", "0")))
    res = run_bass_kernel_spmd(nc, in_maps, core_ids=list(range(NCORES)),
                               trace=trace)
    kernel.last_exec_time_ns = res.exec_time_ns
    kernel.last_result = res
    total = sum(float(res.results[k]["out"][0, 0]) for k in range(NCORES))
    return np.array(total, dtype=np.float32)


kernel.last_exec_time_ns = None
kernel.last_result = None
